# revision 16
# baseline (speedup 1.0000x reference)
"""GANLoss kernel for Trainium2: out = -sum_i prob[i, target[i]] * reward[i].

Shapes: prob (8192, 32000) f32, target (8192,) int64, reward (8192,) f32.
Sharding: rows split across 8 NeuronCores (1024 rows/core).

Per-core pipeline (all on the gpsimd/Pool engine):
 1. One DMA stages a [128, 336] uint16 metadata tile into SBUF: per-call
    int16 gather indices, per-group uint16 select positions, a reward-valued
    one-hot mask (f32), and identity int16 scatter indices.
 2. 8 dma_gather calls fetch, for each of the 1024 rows, the 512B chunk of
    its prob row containing the target element (row 128g+p -> partition p,
    slot g; chunk index = p*250 + target//128, int16-safe).
 3. indirect_copy selects 128 candidate elements per partition (each
    16-partition group shares its union index list; a partition's own picks
    sit at columns i with i%16 == p%16).
 4. tensor_tensor multiplies by the mask, which holds reward at own-pick
    cells and 0 elsewhere - fusing the reward multiply with junk removal.
 5. dma_scatter_add (identity indices, elem 128) lands each partition's
    128-wide masked products in its own row of the zeroed output.
The host sums the 8 cores' [128, 128] partials (junk cells are exact
zeros) and negates.
"""

import numpy as np

N, C = 8192, 32000
N_CORES = 8
ROWS_PER_CORE = N // N_CORES          # 1024
P = 128                               # partitions
S = ROWS_PER_CORE // P                # 8 row-slots per partition / gather calls
ELEM = 128                            # chunk width (512B)
CPR = C // ELEM                       # 250 chunks per row
MW = 336                              # meta width in uint16

# meta layout (uint16 columns)
GIDX0, GIDX1 = 0, 8 * S               # 8 calls x [128, 8] int16
CIDX0, CIDX1 = 64, 72                 # icopy positions, uint16
MASK0, MASK1 = 72, 328                # [128, 128] f32 reward mask
SIDX0, SIDX1 = 328, 336               # scatter identity, int16

_cached = None


def _build_bass():
    import concourse.bacc as bacc
    import concourse.mybir as mybir
    from concourse import library_config

    f32 = mybir.dt.float32
    i16 = mybir.dt.int16
    u16 = mybir.dt.uint16

    nc = bacc.Bacc()
    prob_d = nc.declare_dram_parameter("prob", [ROWS_PER_CORE, C], f32, isOutput=False)
    meta_d = nc.declare_dram_parameter("meta", [P, MW], u16, isOutput=False)
    out_d = nc.declare_dram_parameter("out", [P, ELEM], f32, isOutput=True)

    with (
        nc.sbuf_tensor([P, MW], u16) as meta_sb,
        nc.sbuf_tensor([P, S, 1, ELEM], f32) as gath_sb,
        nc.sbuf_tensor([P, ELEM], f32) as sel_sb,
        nc.sbuf_tensor([P, 1, ELEM], f32) as prod_sb,
        nc.semaphore("lsem") as lsem,
        nc.semaphore("gsem") as gsem,
        nc.semaphore("csem") as csem,
        nc.semaphore("vsem") as vsem,
        nc.semaphore("osem") as osem,
    ):
        g = nc.gpsimd
        g.dma_start(meta_sb[:], meta_d[:]).then_inc(lsem, 16)
        g.wait_ge(lsem, 16)
        g.load_library(library_config.mlp)
        for call in range(S):
            src = prob_d[P * call : P * (call + 1), :].rearrange(
                "r (c e) -> (r c) e", e=ELEM
            )
            g.dma_gather(
                gath_sb[:, call],
                src,
                meta_sb[:, 8 * call : 8 * (call + 1)].bitcast(i16),
                num_idxs=P,
                num_idxs_reg=P,
                elem_size=ELEM,
            ).then_inc(gsem, 16)
        g.wait_ge(gsem, 16 * S)
        g.indirect_copy(
            sel_sb[:],
            gath_sb[:].rearrange("p a b c -> p (a b c)"),
            meta_sb[:, CIDX0:CIDX1],
            i_know_ap_gather_is_preferred=True,
        ).then_inc(csem, 1)
        g.load_library(library_config.standard)
        g.wait_ge(csem, 1)
        g.tensor_tensor(
            out=prod_sb[:, 0, :],
            in0=sel_sb[:],
            in1=meta_sb[:, MASK0:MASK1].bitcast(f32),
            op=mybir.AluOpType.mult,
        ).then_inc(vsem, 1)
        g.load_library(library_config.mlp)
        g.wait_ge(vsem, 1)
        g.dma_scatter_add(
            out_ap=out_d[:],
            in_ap=prod_sb[:],
            idxs_ap=meta_sb[:, SIDX0:SIDX1].bitcast(i16),
            num_idxs=P,
            num_idxs_reg=P,
            elem_size=ELEM,
        ).then_inc(osem, 16)
        g.wait_ge(osem, 16)

    nc.compile()
    return nc


def _shard_host_inputs(prob, target, reward):
    """Per-core in_maps: prob shard + packed uint16 metadata tile."""
    t_all = np.asarray(target).astype(np.int64)
    r_all = np.asarray(reward).astype(np.float32)
    prob = np.asarray(prob, dtype=np.float32)

    # identity scatter indices, wrapped (value(ch, s) = s*16 + ch), tiled x8
    ch, s = np.meshgrid(np.arange(16), np.arange(S), indexing="ij")
    ident16 = (s * 16 + ch).astype(np.int16)                    # (16, 8)
    sidx_u16 = np.tile(ident16, (8, 1)).view(np.uint16)         # (128, 8)

    in_maps = []
    for core in range(N_CORES):
        base = core * ROWS_PER_CORE
        t = t_all[base : base + ROWS_PER_CORE]                  # (1024,)
        r = r_all[base : base + ROWS_PER_CORE]

        meta = np.zeros((P, MW), np.uint16)

        # gather indices: call g, idx# k -> partition k holds row 128g+k's
        # chunk; wrapped [16, 8] with value(ch, s2) = idx#(s2*16+ch), tiled x8
        for call in range(S):
            rows = t[P * call : P * (call + 1)]                 # targets of rows 128g+k
            idxv = (np.arange(P) * CPR + rows // ELEM).astype(np.int16)  # (128,)
            wrapped = idxv.reshape(S, 16).T                     # [ch, s2] = idx#(s2*16+ch)
            meta[:, 8 * call : 8 * (call + 1)] = np.tile(
                wrapped.view(np.uint16), (8, 1)
            )

        # icopy positions: group q's list item i = s*128 + t(128s+16q+i%16)%128
        # (s = i//16); stored wrapped: value(ch2, s2) at row 16q+ch2, col s2
        cidx = np.zeros((P, S), np.uint16)
        for q in range(8):
            for ch2 in range(16):
                for s2 in range(S):
                    row = 128 * s2 + 16 * q + ch2
                    cidx[16 * q + ch2, s2] = s2 * ELEM + (t[row] % ELEM)
        meta[:, CIDX0:CIDX1] = cidx

        # reward-valued one-hot mask [128, 128] f32
        mask = np.zeros((P, ELEM), np.float32)
        pp, ii = np.meshgrid(np.arange(P), np.arange(ELEM), indexing="ij")
        own = (ii % 16) == (pp % 16)
        rowsel = 128 * (ii // 16) + pp
        mask[own] = r[rowsel[own]]
        meta[:, MASK0:MASK1] = mask.view(np.uint16)

        meta[:, SIDX0:SIDX1] = sidx_u16

        # "out" seeds the zero-initialized output buffer for simulators that
        # mark unwritten memory (the PJRT/NRT runtimes donate zeroed buffers);
        # runners that only bind declared ExternalInputs ignore this key.
        in_maps.append(
            {
                "prob": prob[base : base + ROWS_PER_CORE],
                "meta": meta,
                "out": np.zeros((P, ELEM), np.float32),
            }
        )
    return in_maps


def kernel(prob, target, reward):
    global _cached
    from concourse.bass_utils import run_bass_kernel_spmd

    if _cached is None:
        _cached = _build_bass()
    nc = _cached
    in_maps = _shard_host_inputs(prob, target, reward)
    res = run_bass_kernel_spmd(nc, in_maps, list(range(N_CORES)))
    total = np.float64(0.0)
    for core_out in res.results:
        total += np.asarray(core_out["out"], dtype=np.float64).sum()
    return np.float32(-total)
